# revision 14
# baseline (speedup 1.0000x reference)
"""MoE SwiGLU experts (T=2048, H=2048, I=5632, E=8, top-2) on 8 trn2 cores.

Strategy: expert-parallel routed compute in bf16. The reference computes
all 8 experts densely for every token, but the output only needs each
token's top-2 experts, so we gather tokens per expert on the host
(merging the case where both top-k slots pick the same expert), run one
expert per NeuronCore on its ~T*K/E gathered tokens, and scatter-combine
with the router weights on the host.  4x less device FLOPs than dense.

bf16 matmuls run at the same 1 column/cycle PE rate as float32r but:
  - halve HBM weight traffic (69 MB/core vs 138) so DMA never paces PE,
  - get fast weight loads (FWL) so LDWEIGHTS hides under the matmul,
  - halve SBUF footprint.
Measured end-to-end bf16 error vs the fp32 reference is ~4e-3
max-normalized (gate is 2e-2).

Per core (expert e), with C = padded token capacity (C=512 nominal):
  phase 1: hT[i, c] = silu(w1[e].T @ xgT) * (w3[e].T @ xgT)   [I, C]
           16 H-chunks of 128 accumulated in PSUM; per icg (256 wide
           in I) only 4 PSUM banks are used so consecutive icgs
           alternate bank sets and the PE never waits on PSUM drains.
  phase 2: y[c, h]  = hT.T @ w2[e]                            [C, H]
           44 I-chunks of 128 accumulated in PSUM; 4 banks per
           512-wide output group, same alternation.
Weights are host-retiled so every DMA is one contiguous [128, 4KB]
block, and the first icg's weights are issued interleaved with the
gathered tokens so the first matmul chain starts within ~2us.
"""

import numpy as np
import ml_dtypes

import concourse.bacc as bacc
import concourse.mybir as mybir
import concourse.tile as tile
from concourse.bass_utils import run_bass_kernel_spmd

E = 8
H = 2048
I = 5632
HK = H // 128    # 16 contraction chunks for phase 1
IK = I // 128    # 44 contraction chunks for phase 2
HG = H // 512    # 4 output column groups (w2)
C_CAP = 640      # max tokens per expert per round (PSUM-chunk budget)

F32 = mybir.dt.float32
BF16 = mybir.dt.bfloat16
NP_BF16 = ml_dtypes.bfloat16
SILU = mybir.ActivationFunctionType.Silu

_prog_cache: dict[int, object] = {}


def _chunk_list(c):
    """Split the token dim into PSUM-bank-sized (<=512) moving chunks."""
    out, off = [], 0
    while off < c:
        t = min(512, c - off)
        out.append((off, t))
        off += t
    return out


WARMUP_MM = 8


def _build(c):
    nc = bacc.Bacc("TRN2", target_bir_lowering=False, debug=False, num_devices=E)
    ch = _chunk_list(c)
    nch = len(ch)
    tt_n = -(-c // 128)
    # w1/w3 merged: [icg, q, 128, 2(w), 4(hk%4), 256] -> 4KB/partition DMAs
    w13 = nc.dram_tensor(
        "w13", [I // 256, HK // 4, 128, 2, 4, 256], BF16, kind="ExternalInput"
    )
    # w2: [hg, kq, 128, 4(ik%4), 512] -> 4KB/partition DMAs
    w2 = nc.dram_tensor(
        "w2", [HG, IK // 4, 128, 4, 512], BF16, kind="ExternalInput"
    )
    xgT = nc.dram_tensor("xgT", [HK, 128, c], BF16, kind="ExternalInput")
    y = nc.dram_tensor("y", [c, H], BF16, kind="ExternalOutput")
    scratch = nc.dram_tensor("scratch", [128, 512], F32, kind="ExternalOutput")

    n_icg = I // 256

    with tile.TileContext(nc) as tc:
        with (
            tc.tile_pool(name="xg", bufs=1) as xpool,
            tc.tile_pool(name="h", bufs=1) as hpool,
            tc.tile_pool(name="w", bufs=6) as wpool,
            tc.tile_pool(name="w2p", bufs=4) as w2pool,
            tc.tile_pool(name="ps", bufs=8, space="PSUM") as pspool,
            tc.tile_pool(name="o", bufs=4) as opool,
        ):
            # Input DMAs are issued FIRST so the sync queue dispatches
            # them at t=0 (nothing queued ahead).  The first icg's weight
            # blocks interleave with the xg stream so hk=0..3 matmuls
            # unblock early instead of the weights queueing behind all
            # of xg.
            xg = []
            for hk in range(HK):
                t = xpool.tile([128, c], BF16, tag=f"xg{hk}", name=f"xg{hk}")
                xg.append(t)
            w13_0 = []
            for q in range(HK // 4):
                nc.sync.dma_start(xg[4 * q][:], xgT[4 * q])
                wt = wpool.tile([128, 2, 4, 256], BF16, tag="w", name=f"w13_0_{q}")
                nc.sync.dma_start(wt[:], w13[0, q])
                w13_0.append(wt)
                for j in range(1, 4):
                    nc.sync.dma_start(xg[4 * q + j][:], xgT[4 * q + j])

            # PE warmup: short matmuls on a zeroed tile cover the HAM
            # clock ramp + first-input DMA latency.  The scratch DMA that
            # keeps them live goes out on the scalar queue so it never
            # blocks input-DMA dispatch on the sync queue.
            wu = xpool.tile([128, 512], BF16, tag="wu", name="wu")
            nc.vector.memset(wu[:], 0.0)
            wups = pspool.tile([128, 512], F32, tag="ps", name="wups")
            for _ in range(WARMUP_MM):
                nc.tensor.matmul(wups[:, :256], wu[:, :128], wu[:, :256],
                                 start=True, stop=True)
            wuo = opool.tile([128, 512], F32, tag="o", name="wuo")
            nc.vector.tensor_copy(wuo[:, :256], wups[:, :256])
            nc.scalar.dma_start(scratch[:, :256], wuo[:, :256])
            hT = [
                hpool.tile([128, c], BF16, tag=f"h{ik}", name=f"h{ik}")
                for ik in range(IK)
            ]

            # phase 1: hT = silu(w1.T @ xgT) * (w3.T @ xgT)
            # 2(w) * 2(ic) * nch PSUM groups per icg; 4 when c<=512 so
            # bank sets alternate across icgs.
            for icg in range(n_icg):
                ps = {}
                for w in (0, 1):
                    for ic in (0, 1):
                        for ci in range(nch):
                            ps[w, ic, ci] = pspool.tile(
                                [128, ch[ci][1]], F32, tag="ps",
                                name=f"ps{icg}_{w}_{ic}_{ci}",
                            )
                for q in range(HK // 4):
                    if icg == 0:
                        wt = w13_0[q]
                    else:
                        wt = wpool.tile(
                            [128, 2, 4, 256], BF16, tag="w", name=f"w13_{icg}_{q}"
                        )
                        nc.sync.dma_start(wt[:], w13[icg, q])
                    for j in range(4):
                        hk = 4 * q + j
                        for w in (0, 1):
                            for ic in (0, 1):
                                for ci, (off, sz) in enumerate(ch):
                                    nc.tensor.matmul(
                                        ps[w, ic, ci][:],
                                        wt[:, w, j, ic * 128 : (ic + 1) * 128],
                                        xg[hk][:, off : off + sz],
                                        start=(hk == 0),
                                        stop=(hk == HK - 1),
                                    )
                for ic in (0, 1):
                    ik = icg * 2 + ic
                    for ci, (off, sz) in enumerate(ch):
                        dst = hT[ik][:, off : off + sz]
                        nc.scalar.activation(dst, ps[0, ic, ci][:], SILU)
                        nc.vector.tensor_mul(dst, dst, ps[1, ic, ci][:])

            # phase 2: y = hT.T @ w2, one 512-wide output group at a
            # time so only tt_n PSUM banks are held and the drain of
            # group g overlaps the matmuls of group g+1.
            for hg in range(HG):
                ps2 = [
                    pspool.tile([128, 512], F32, tag="ps", name=f"ps2_{hg}_{tt}")
                    for tt in range(tt_n)
                ]
                for kq in range(IK // 4):
                    wt = w2pool.tile(
                        [128, 4, 512], BF16, tag="w2", name=f"w2t_{hg}_{kq}"
                    )
                    nc.sync.dma_start(wt[:], w2[hg, kq])
                    for j in range(4):
                        ik = 4 * kq + j
                        for tt in range(tt_n):
                            t0 = tt * 128
                            t1 = min(t0 + 128, c)
                            nc.tensor.matmul(
                                ps2[tt][: t1 - t0, :],
                                hT[ik][:, t0:t1],
                                wt[:, j, :],
                                start=(ik == 0),
                                stop=(ik == IK - 1),
                            )
                for tt in range(tt_n):
                    t0 = tt * 128
                    t1 = min(t0 + 128, c)
                    ot = opool.tile([128, 512], BF16, tag="ob", name=f"o{hg}_{tt}")
                    # alternate copy engines so the final drain is not
                    # serialized on the vector engine
                    if tt % 2 == 0:
                        nc.vector.tensor_copy(ot[: t1 - t0, :], ps2[tt][: t1 - t0, :])
                    else:
                        nc.scalar.copy(ot[: t1 - t0, :], ps2[tt][: t1 - t0, :])
                    nc.sync.dma_start(
                        y[t0:t1, hg * 512 : (hg + 1) * 512], ot[: t1 - t0, :]
                    )
    nc.compile()
    return nc


def _get_prog(c):
    if c not in _prog_cache:
        _prog_cache[c] = _build(c)
    return _prog_cache[c]


def _retile_weights(w1, w2, w3):
    """Host retiling (f32 -> bf16) so every device DMA is one contiguous
    [128, 4KB] block."""
    b = lambda a: np.ascontiguousarray(a).astype(NP_BF16)
    # w13[e, icg, q, p, w, j, i] = w{1,3}[e, (q*4+j)*128 + p, icg*256 + i]
    w1r = w1.reshape(E, HK // 4, 4, 128, I // 256, 256).transpose(0, 4, 1, 3, 2, 5)
    w3r = w3.reshape(E, HK // 4, 4, 128, I // 256, 256).transpose(0, 4, 1, 3, 2, 5)
    w13 = b(np.stack([w1r, w3r], axis=4))
    # w2t[e, hg, kq, p, j, h] = w2[e, (kq*4+j)*128 + p, hg*512 + h]
    w2t = b(w2.reshape(E, IK // 4, 4, 128, HG, 512).transpose(0, 4, 1, 3, 2, 5))
    return w13, w2t


def kernel(x, expert_weights, w1, w2, w3, expert_indices):
    x = np.asarray(x, dtype=np.float32)
    expert_weights = np.asarray(expert_weights, dtype=np.float32)
    w1 = np.asarray(w1, dtype=np.float32)
    w2 = np.asarray(w2, dtype=np.float32)
    w3 = np.asarray(w3, dtype=np.float32)
    idx = np.asarray(expert_indices)
    T = x.shape[0]

    # Route: token lists per expert, merging duplicate top-k hits so each
    # token appears at most once per expert (scatter-add safe).
    same = idx[:, 0] == idx[:, 1]
    w_slot0 = np.where(same, expert_weights[:, 0] + expert_weights[:, 1],
                       expert_weights[:, 0])
    toks, wts = [], []
    for e in range(E):
        m0 = idx[:, 0] == e
        m1 = (idx[:, 1] == e) & ~same
        t0 = np.nonzero(m0)[0]
        t1 = np.nonzero(m1)[0]
        toks.append(np.concatenate([t0, t1]))
        wts.append(np.concatenate([w_slot0[m0], expert_weights[m1, 1]]))

    maxcount = max(len(t) for t in toks)
    maxcount = max(maxcount, 1)
    nrounds = -(-maxcount // C_CAP)
    # bf16 matmuls run at 1 col/cycle for any moving size, so the token
    # capacity only needs 8-alignment (16B DMA lines), not 128.
    c = -(-(-(-maxcount // nrounds)) // 8) * 8
    c = max(c, 128)

    w13t, w2t = _retile_weights(w1, w2, w3)
    nc = _get_prog(c)

    xb = x.T.astype(NP_BF16)  # [H, T] once, sliced per expert below
    out = np.zeros((T, H), dtype=np.float32)
    for r in range(nrounds):
        in_maps = []
        seg_toks = []
        seg_wts = []
        for e in range(E):
            seg = toks[e][r * c : (r + 1) * c]
            sw = wts[e][r * c : (r + 1) * c]
            seg_toks.append(seg)
            seg_wts.append(sw)
            xga = np.zeros((H, c), dtype=NP_BF16)
            if len(seg):
                xga[:, : len(seg)] = xb[:, seg]
            in_maps.append(
                {
                    "xgT": np.ascontiguousarray(xga.reshape(HK, 128, c)),
                    "w13": w13t[e],
                    "w2": w2t[e],
                }
            )
        res = run_bass_kernel_spmd(nc, in_maps, core_ids=list(range(E)))
        for e in range(E):
            seg = seg_toks[e]
            if len(seg) == 0:
                continue
            ye = res.results[e]["y"][: len(seg)].astype(np.float32)
            out[seg] += ye * seg_wts[e][:, None]
    return out


# revision 18
# speedup vs baseline: 1.0022x; 1.0022x over previous
"""MoE SwiGLU experts (T=2048, H=2048, I=5632, E=8, top-2) on 8 trn2 cores.

Strategy: expert-parallel routed compute in bf16. The reference computes
all 8 experts densely for every token, but the output only needs each
token's top-2 experts, so we gather tokens per expert on the host
(merging the case where both top-k slots pick the same expert), run one
expert per NeuronCore on its ~T*K/E gathered tokens, and scatter-combine
with the router weights on the host.  4x less device FLOPs than dense.

bf16 matmuls run at the same 1 column/cycle PE rate as float32r but:
  - halve HBM weight traffic (69 MB/core vs 138) so DMA never paces PE,
  - get fast weight loads (FWL) so LDWEIGHTS hides under the matmul,
  - halve SBUF footprint.
Measured end-to-end bf16 error vs the fp32 reference is ~4e-3
max-normalized (gate is 2e-2).

Per core (expert e), with C = padded token capacity (C=512 nominal):
  phase 1: hT[i, c] = silu(w1[e].T @ xgT) * (w3[e].T @ xgT)   [I, C]
           16 H-chunks of 128 accumulated in PSUM; per icg (256 wide
           in I) only 4 PSUM banks are used so consecutive icgs
           alternate bank sets and the PE never waits on PSUM drains.
  phase 2: y[c, h]  = hT.T @ w2[e]                            [C, H]
           44 I-chunks of 128 accumulated in PSUM; 4 banks per
           512-wide output group, same alternation.
Weights are host-retiled so every DMA is one contiguous [128, 4KB]
block, and the first icg's weights are issued interleaved with the
gathered tokens so the first matmul chain starts within ~2us.
"""

import numpy as np
import ml_dtypes

import concourse.bacc as bacc
import concourse.mybir as mybir
import concourse.tile as tile
from concourse.bass_utils import run_bass_kernel_spmd

E = 8
H = 2048
I = 5632
HK = H // 128    # 16 contraction chunks for phase 1
IK = I // 128    # 44 contraction chunks for phase 2
HG = H // 512    # 4 output column groups (w2)
C_CAP = 640      # max tokens per expert per round (PSUM-chunk budget)

F32 = mybir.dt.float32
BF16 = mybir.dt.bfloat16
NP_BF16 = ml_dtypes.bfloat16
SILU = mybir.ActivationFunctionType.Silu

_prog_cache: dict[int, object] = {}


def _chunk_list(c):
    """Split the token dim into PSUM-bank-sized (<=512) moving chunks."""
    out, off = [], 0
    while off < c:
        t = min(512, c - off)
        out.append((off, t))
        off += t
    return out


WARMUP_MM = 8


def _build(c):
    nc = bacc.Bacc("TRN2", target_bir_lowering=False, debug=False, num_devices=E)
    ch = _chunk_list(c)
    nch = len(ch)
    tt_n = -(-c // 128)
    # w1/w3 merged: [icg, q, 128, 2(w), 4(hk%4), 256] -> 4KB/partition DMAs
    w13 = nc.dram_tensor(
        "w13", [I // 256, HK // 4, 128, 2, 4, 256], BF16, kind="ExternalInput"
    )
    # w2: [sec, kq, 128, 4(ik%4), 4(h-tile), 128] -> 4KB/partition DMAs
    w2 = nc.dram_tensor(
        "w2", [HG, IK // 4, 128, 4, 4, 128], BF16, kind="ExternalInput"
    )
    xgT = nc.dram_tensor("xgT", [HK, 128, c], BF16, kind="ExternalInput")
    y = nc.dram_tensor("y", [H, c], BF16, kind="ExternalOutput")
    scratch = nc.dram_tensor("scratch", [128, 512], F32, kind="ExternalOutput")

    n_icg = I // 256

    with tile.TileContext(nc) as tc:
        with (
            tc.tile_pool(name="xg", bufs=1) as xpool,
            tc.tile_pool(name="h", bufs=1) as hpool,
            tc.tile_pool(name="w", bufs=6) as wpool,
            tc.tile_pool(name="w2p", bufs=4) as w2pool,
            tc.tile_pool(name="ps", bufs=8, space="PSUM") as pspool,
            tc.tile_pool(name="o", bufs=4) as opool,
        ):
            # Input DMAs are issued FIRST so the sync queue dispatches
            # them at t=0 (nothing queued ahead).  The first icg's weight
            # blocks interleave with the xg stream so hk=0..3 matmuls
            # unblock early instead of the weights queueing behind all
            # of xg.
            xg = []
            for hk in range(HK):
                t = xpool.tile([128, c], BF16, tag=f"xg{hk}", name=f"xg{hk}")
                xg.append(t)
            w13_0 = []
            for q in range(HK // 4):
                nc.sync.dma_start(xg[4 * q][:], xgT[4 * q])
                wt = wpool.tile([128, 2, 4, 256], BF16, tag="w", name=f"w13_0_{q}")
                nc.sync.dma_start(wt[:], w13[0, q])
                w13_0.append(wt)
                for j in range(1, 4):
                    nc.sync.dma_start(xg[4 * q + j][:], xgT[4 * q + j])

            # PE warmup: short matmuls on a zeroed tile cover the HAM
            # clock ramp + first-input DMA latency.  The scratch DMA that
            # keeps them live goes out on the scalar queue so it never
            # blocks input-DMA dispatch on the sync queue.
            wu = xpool.tile([128, 512], BF16, tag="wu", name="wu")
            nc.vector.memset(wu[:], 0.0)
            wups = pspool.tile([128, 512], F32, tag="ps", name="wups")
            for _ in range(WARMUP_MM):
                nc.tensor.matmul(wups[:, :256], wu[:, :128], wu[:, :256],
                                 start=True, stop=True)
            wuo = opool.tile([128, 512], F32, tag="o", name="wuo")
            nc.vector.tensor_copy(wuo[:, :256], wups[:, :256])
            nc.scalar.dma_start(scratch[:, :256], wuo[:, :256])
            hT = [
                hpool.tile([128, c], BF16, tag=f"h{ik}", name=f"h{ik}")
                for ik in range(IK)
            ]

            # phase 1: hT = silu(w1.T @ xgT) * (w3.T @ xgT)
            # 2(w) * 2(ic) * nch PSUM groups per icg; 4 when c<=512 so
            # bank sets alternate across icgs.
            for icg in range(n_icg):
                ps = {}
                for w in (0, 1):
                    for ic in (0, 1):
                        for ci in range(nch):
                            ps[w, ic, ci] = pspool.tile(
                                [128, ch[ci][1]], F32, tag="ps",
                                name=f"ps{icg}_{w}_{ic}_{ci}",
                            )
                for q in range(HK // 4):
                    if icg == 0:
                        wt = w13_0[q]
                    else:
                        wt = wpool.tile(
                            [128, 2, 4, 256], BF16, tag="w", name=f"w13_{icg}_{q}"
                        )
                        nc.sync.dma_start(wt[:], w13[icg, q])
                    for j in range(4):
                        hk = 4 * q + j
                        for w in (0, 1):
                            for ic in (0, 1):
                                for ci, (off, sz) in enumerate(ch):
                                    nc.tensor.matmul(
                                        ps[w, ic, ci][:],
                                        wt[:, w, j, ic * 128 : (ic + 1) * 128],
                                        xg[hk][:, off : off + sz],
                                        start=(hk == 0),
                                        stop=(hk == HK - 1),
                                    )
                for ic in (0, 1):
                    ik = icg * 2 + ic
                    for ci, (off, sz) in enumerate(ch):
                        dst = hT[ik][:, off : off + sz]
                        nc.scalar.activation(dst, ps[0, ic, ci][:], SILU)
                        nc.vector.tensor_mul(dst, dst, ps[1, ic, ci][:])

            # phase 2: yT = w2.T @ hT with the (shorter) token dim as the
            # moving operand.  One 512-wide output-row section at a time
            # so only 4*nch PSUM banks are held and the drain of section
            # s overlaps the matmuls of section s+1.
            for sec in range(HG):
                ps2 = {
                    (ht, ci): pspool.tile(
                        [128, ch[ci][1]], F32, tag="ps", name=f"ps2_{sec}_{ht}_{ci}"
                    )
                    for ht in range(4)
                    for ci in range(nch)
                }
                for kq in range(IK // 4):
                    wt = w2pool.tile(
                        [128, 4, 4, 128], BF16, tag="w2", name=f"w2t_{sec}_{kq}"
                    )
                    nc.sync.dma_start(wt[:], w2[sec, kq])
                    for j in range(4):
                        ik = 4 * kq + j
                        for ht in range(4):
                            for ci, (off, sz) in enumerate(ch):
                                nc.tensor.matmul(
                                    ps2[ht, ci][:],
                                    wt[:, j, ht, :],
                                    hT[ik][:, off : off + sz],
                                    start=(ik == 0),
                                    stop=(ik == IK - 1),
                                )
                for ht in range(4):
                    ot = opool.tile([128, c], BF16, tag="ob", name=f"o{sec}_{ht}")
                    for ci, (off, sz) in enumerate(ch):
                        # alternate copy engines so the final drain is
                        # not serialized on the vector engine
                        if ht % 2 == 0:
                            nc.vector.tensor_copy(
                                ot[:, off : off + sz], ps2[ht, ci][:]
                            )
                        else:
                            nc.scalar.copy(ot[:, off : off + sz], ps2[ht, ci][:])
                    r0 = sec * 512 + ht * 128
                    nc.sync.dma_start(y[r0 : r0 + 128, :], ot[:])
    nc.compile()
    return nc


def _get_prog(c):
    if c not in _prog_cache:
        _prog_cache[c] = _build(c)
    return _prog_cache[c]


def _retile_weights(w1, w2, w3):
    """Host retiling (f32 -> bf16) so every device DMA is one contiguous
    [128, 4KB] block."""
    b = lambda a: np.ascontiguousarray(a).astype(NP_BF16)
    # w13[e, icg, q, p, w, j, i] = w{1,3}[e, (q*4+j)*128 + p, icg*256 + i]
    w1r = w1.reshape(E, HK // 4, 4, 128, I // 256, 256).transpose(0, 4, 1, 3, 2, 5)
    w3r = w3.reshape(E, HK // 4, 4, 128, I // 256, 256).transpose(0, 4, 1, 3, 2, 5)
    w13 = b(np.stack([w1r, w3r], axis=4))
    # w2t[e, sec, kq, p, j, ht, hc] = w2[e, (kq*4+j)*128+p, sec*512+ht*128+hc]
    w2t = b(
        w2.reshape(E, IK // 4, 4, 128, HG, 4, 128).transpose(0, 4, 1, 3, 2, 5, 6)
    )
    return w13, w2t


def kernel(x, expert_weights, w1, w2, w3, expert_indices):
    x = np.asarray(x, dtype=np.float32)
    expert_weights = np.asarray(expert_weights, dtype=np.float32)
    w1 = np.asarray(w1, dtype=np.float32)
    w2 = np.asarray(w2, dtype=np.float32)
    w3 = np.asarray(w3, dtype=np.float32)
    idx = np.asarray(expert_indices)
    T = x.shape[0]

    # Route: token lists per expert, merging duplicate top-k hits so each
    # token appears at most once per expert (scatter-add safe).
    same = idx[:, 0] == idx[:, 1]
    w_slot0 = np.where(same, expert_weights[:, 0] + expert_weights[:, 1],
                       expert_weights[:, 0])
    toks, wts = [], []
    for e in range(E):
        m0 = idx[:, 0] == e
        m1 = (idx[:, 1] == e) & ~same
        t0 = np.nonzero(m0)[0]
        t1 = np.nonzero(m1)[0]
        toks.append(np.concatenate([t0, t1]))
        wts.append(np.concatenate([w_slot0[m0], expert_weights[m1, 1]]))

    maxcount = max(len(t) for t in toks)
    maxcount = max(maxcount, 1)
    nrounds = -(-maxcount // C_CAP)
    # bf16 matmuls run at 1 col/cycle for any moving size, so the token
    # capacity only needs 8-alignment (16B DMA lines), not 128.
    c = -(-(-(-maxcount // nrounds)) // 8) * 8
    c = max(c, 128)

    w13t, w2t = _retile_weights(w1, w2, w3)
    nc = _get_prog(c)

    xb = x.T.astype(NP_BF16)  # [H, T] once, sliced per expert below
    out = np.zeros((T, H), dtype=np.float32)
    for r in range(nrounds):
        in_maps = []
        seg_toks = []
        seg_wts = []
        for e in range(E):
            seg = toks[e][r * c : (r + 1) * c]
            sw = wts[e][r * c : (r + 1) * c]
            seg_toks.append(seg)
            seg_wts.append(sw)
            xga = np.zeros((H, c), dtype=NP_BF16)
            if len(seg):
                xga[:, : len(seg)] = xb[:, seg]
            in_maps.append(
                {
                    "xgT": np.ascontiguousarray(xga.reshape(HK, 128, c)),
                    "w13": w13t[e],
                    "w2": w2t[e],
                }
            )
        res = run_bass_kernel_spmd(nc, in_maps, core_ids=list(range(E)))
        for e in range(E):
            seg = seg_toks[e]
            if len(seg) == 0:
                continue
            ye = res.results[e]["y"][:, : len(seg)].T.astype(np.float32)
            out[seg] += ye * seg_wts[e][:, None]
    return out


# revision 20
# speedup vs baseline: 1.0039x; 1.0018x over previous
"""MoE SwiGLU experts (T=2048, H=2048, I=5632, E=8, top-2) on 8 trn2 cores.

Strategy: expert-parallel routed compute in bf16. The reference computes
all 8 experts densely for every token, but the output only needs each
token's top-2 experts, so we gather tokens per expert on the host
(merging the case where both top-k slots pick the same expert), run one
expert per NeuronCore on its ~T*K/E gathered tokens, and scatter-combine
with the router weights on the host.  4x less device FLOPs than dense.

bf16 matmuls run at the same 1 column/cycle PE rate as float32r but:
  - halve HBM weight traffic (69 MB/core vs 138) so DMA never paces PE,
  - get fast weight loads (FWL) so LDWEIGHTS hides under the matmul,
  - halve SBUF footprint.
Measured end-to-end bf16 error vs the fp32 reference is ~4e-3
max-normalized (gate is 2e-2).

Per core (expert e), with C = padded token capacity (C=512 nominal):
  phase 1: hT[i, c] = silu(w1[e].T @ xgT) * (w3[e].T @ xgT)   [I, C]
           16 H-chunks of 128 accumulated in PSUM; per icg (256 wide
           in I) only 4 PSUM banks are used so consecutive icgs
           alternate bank sets and the PE never waits on PSUM drains.
  phase 2: y[c, h]  = hT.T @ w2[e]                            [C, H]
           44 I-chunks of 128 accumulated in PSUM; 4 banks per
           512-wide output group, same alternation.
Weights are host-retiled so every DMA is one contiguous [128, 4KB]
block, and the first icg's weights are issued interleaved with the
gathered tokens so the first matmul chain starts within ~2us.
"""

import numpy as np
import ml_dtypes

import concourse.bacc as bacc
import concourse.mybir as mybir
import concourse.tile as tile
from concourse.bass_utils import run_bass_kernel_spmd

E = 8
H = 2048
I = 5632
HK = H // 128    # 16 contraction chunks for phase 1
IK = I // 128    # 44 contraction chunks for phase 2
HG = H // 512    # 4 output column groups (w2)
C_CAP = 640      # max tokens per expert per round (PSUM-chunk budget)

F32 = mybir.dt.float32
BF16 = mybir.dt.bfloat16
NP_BF16 = ml_dtypes.bfloat16
SILU = mybir.ActivationFunctionType.Silu

_prog_cache: dict[int, object] = {}


def _chunk_list(c):
    """Split the token dim into PSUM-bank-sized (<=512) moving chunks."""
    out, off = [], 0
    while off < c:
        t = min(512, c - off)
        out.append((off, t))
        off += t
    return out


WARMUP_MM = 8


def _build(c):
    nc = bacc.Bacc("TRN2", target_bir_lowering=False, debug=False, num_devices=E)
    ch = _chunk_list(c)
    nch = len(ch)
    tt_n = -(-c // 128)
    # w1/w3 merged: [icg, q, 128, 2(w), 4(hk%4), 256] -> 4KB/partition DMAs
    w13 = nc.dram_tensor(
        "w13", [I // 256, HK // 4, 128, 2, 4, 256], BF16, kind="ExternalInput"
    )
    # w2: [sec, kq, 128, 4(ik%4), 4(h-tile), 128] -> 4KB/partition DMAs
    w2 = nc.dram_tensor(
        "w2", [HG, IK // 4, 128, 4, 4, 128], BF16, kind="ExternalInput"
    )
    xgT = nc.dram_tensor("xgT", [HK, 128, c], BF16, kind="ExternalInput")
    y = nc.dram_tensor("y", [H, c], BF16, kind="ExternalOutput")
    scratch = nc.dram_tensor("scratch", [128, 512], F32, kind="ExternalOutput")

    n_icg = I // 256

    with tile.TileContext(nc) as tc:
        with (
            tc.tile_pool(name="xg", bufs=1) as xpool,
            tc.tile_pool(name="h", bufs=1) as hpool,
            tc.tile_pool(name="w", bufs=8) as wpool,
            tc.tile_pool(name="w2p", bufs=6) as w2pool,
            tc.tile_pool(name="ps", bufs=8, space="PSUM") as pspool,
            tc.tile_pool(name="o", bufs=4) as opool,
        ):
            # Input DMAs are issued FIRST so the sync queue dispatches
            # them at t=0 (nothing queued ahead).  The first icg's weight
            # blocks interleave with the xg stream so hk=0..3 matmuls
            # unblock early instead of the weights queueing behind all
            # of xg.
            xg = []
            for hk in range(HK):
                t = xpool.tile([128, c], BF16, tag=f"xg{hk}", name=f"xg{hk}")
                xg.append(t)
            w13_0 = []
            for q in range(HK // 4):
                if q == 0:
                    # split so the hk=0 operand completes sooner
                    nc.sync.dma_start(xg[0][0:64, :], xgT[0, 0:64])
                    nc.sync.dma_start(xg[0][64:128, :], xgT[0, 64:128])
                else:
                    nc.sync.dma_start(xg[4 * q][:], xgT[4 * q])
                wt = wpool.tile([128, 2, 4, 256], BF16, tag="w", name=f"w13_0_{q}")
                nc.sync.dma_start(wt[:], w13[0, q])
                w13_0.append(wt)
                for j in range(1, 4):
                    nc.sync.dma_start(xg[4 * q + j][:], xgT[4 * q + j])

            # PE warmup: short matmuls on a zeroed tile cover the HAM
            # clock ramp + first-input DMA latency.  The scratch DMA that
            # keeps them live goes out on the scalar queue so it never
            # blocks input-DMA dispatch on the sync queue.
            wu = xpool.tile([128, 512], BF16, tag="wu", name="wu")
            nc.vector.memset(wu[:], 0.0)
            wups = pspool.tile([128, 512], F32, tag="ps", name="wups")
            for _ in range(WARMUP_MM):
                nc.tensor.matmul(wups[:, :256], wu[:, :128], wu[:, :256],
                                 start=True, stop=True)
            wuo = opool.tile([128, 512], F32, tag="o", name="wuo")
            nc.vector.tensor_copy(wuo[:, :256], wups[:, :256])
            nc.scalar.dma_start(scratch[:, :256], wuo[:, :256])
            hT = [
                hpool.tile([128, c], BF16, tag=f"h{ik}", name=f"h{ik}")
                for ik in range(IK)
            ]

            # phase 1: hT = silu(w1.T @ xgT) * (w3.T @ xgT)
            # 2(w) * 2(ic) * nch PSUM groups per icg; 4 when c<=512 so
            # bank sets alternate across icgs.
            for icg in range(n_icg):
                ps = {}
                for w in (0, 1):
                    for ic in (0, 1):
                        for ci in range(nch):
                            ps[w, ic, ci] = pspool.tile(
                                [128, ch[ci][1]], F32, tag="ps",
                                name=f"ps{icg}_{w}_{ic}_{ci}",
                            )
                for q in range(HK // 4):
                    if icg == 0:
                        wt = w13_0[q]
                    else:
                        wt = wpool.tile(
                            [128, 2, 4, 256], BF16, tag="w", name=f"w13_{icg}_{q}"
                        )
                        nc.sync.dma_start(wt[:], w13[icg, q])
                    for j in range(4):
                        hk = 4 * q + j
                        for w in (0, 1):
                            for ic in (0, 1):
                                for ci, (off, sz) in enumerate(ch):
                                    nc.tensor.matmul(
                                        ps[w, ic, ci][:],
                                        wt[:, w, j, ic * 128 : (ic + 1) * 128],
                                        xg[hk][:, off : off + sz],
                                        start=(hk == 0),
                                        stop=(hk == HK - 1),
                                    )
                for ic in (0, 1):
                    ik = icg * 2 + ic
                    for ci, (off, sz) in enumerate(ch):
                        dst = hT[ik][:, off : off + sz]
                        nc.scalar.activation(dst, ps[0, ic, ci][:], SILU)
                        nc.vector.tensor_mul(dst, dst, ps[1, ic, ci][:])

            # phase 2: yT = w2.T @ hT with the (shorter) token dim as the
            # moving operand.  One 512-wide output-row section at a time
            # so only 4*nch PSUM banks are held and the drain of section
            # s overlaps the matmuls of section s+1.
            for sec in range(HG):
                ps2 = {
                    (ht, ci): pspool.tile(
                        [128, ch[ci][1]], F32, tag="ps", name=f"ps2_{sec}_{ht}_{ci}"
                    )
                    for ht in range(4)
                    for ci in range(nch)
                }
                for kq in range(IK // 4):
                    wt = w2pool.tile(
                        [128, 4, 4, 128], BF16, tag="w2", name=f"w2t_{sec}_{kq}"
                    )
                    nc.sync.dma_start(wt[:], w2[sec, kq])
                    for j in range(4):
                        ik = 4 * kq + j
                        for ht in range(4):
                            for ci, (off, sz) in enumerate(ch):
                                nc.tensor.matmul(
                                    ps2[ht, ci][:],
                                    wt[:, j, ht, :],
                                    hT[ik][:, off : off + sz],
                                    start=(ik == 0),
                                    stop=(ik == IK - 1),
                                )
                for ht in range(4):
                    ot = opool.tile([128, c], BF16, tag="ob", name=f"o{sec}_{ht}")
                    for ci, (off, sz) in enumerate(ch):
                        # alternate copy engines so the final drain is
                        # not serialized on the vector engine
                        if ht % 2 == 0:
                            nc.vector.tensor_copy(
                                ot[:, off : off + sz], ps2[ht, ci][:]
                            )
                        else:
                            nc.scalar.copy(ot[:, off : off + sz], ps2[ht, ci][:])
                    r0 = sec * 512 + ht * 128
                    nc.sync.dma_start(y[r0 : r0 + 128, :], ot[:])
    nc.compile()
    return nc


def _get_prog(c):
    if c not in _prog_cache:
        _prog_cache[c] = _build(c)
    return _prog_cache[c]


def _retile_weights(w1, w2, w3):
    """Host retiling (f32 -> bf16) so every device DMA is one contiguous
    [128, 4KB] block."""
    b = lambda a: np.ascontiguousarray(a).astype(NP_BF16)
    # w13[e, icg, q, p, w, j, i] = w{1,3}[e, (q*4+j)*128 + p, icg*256 + i]
    w1r = w1.reshape(E, HK // 4, 4, 128, I // 256, 256).transpose(0, 4, 1, 3, 2, 5)
    w3r = w3.reshape(E, HK // 4, 4, 128, I // 256, 256).transpose(0, 4, 1, 3, 2, 5)
    w13 = b(np.stack([w1r, w3r], axis=4))
    # w2t[e, sec, kq, p, j, ht, hc] = w2[e, (kq*4+j)*128+p, sec*512+ht*128+hc]
    w2t = b(
        w2.reshape(E, IK // 4, 4, 128, HG, 4, 128).transpose(0, 4, 1, 3, 2, 5, 6)
    )
    return w13, w2t


def kernel(x, expert_weights, w1, w2, w3, expert_indices):
    x = np.asarray(x, dtype=np.float32)
    expert_weights = np.asarray(expert_weights, dtype=np.float32)
    w1 = np.asarray(w1, dtype=np.float32)
    w2 = np.asarray(w2, dtype=np.float32)
    w3 = np.asarray(w3, dtype=np.float32)
    idx = np.asarray(expert_indices)
    T = x.shape[0]

    # Route: token lists per expert, merging duplicate top-k hits so each
    # token appears at most once per expert (scatter-add safe).
    same = idx[:, 0] == idx[:, 1]
    w_slot0 = np.where(same, expert_weights[:, 0] + expert_weights[:, 1],
                       expert_weights[:, 0])
    toks, wts = [], []
    for e in range(E):
        m0 = idx[:, 0] == e
        m1 = (idx[:, 1] == e) & ~same
        t0 = np.nonzero(m0)[0]
        t1 = np.nonzero(m1)[0]
        toks.append(np.concatenate([t0, t1]))
        wts.append(np.concatenate([w_slot0[m0], expert_weights[m1, 1]]))

    maxcount = max(len(t) for t in toks)
    maxcount = max(maxcount, 1)
    nrounds = -(-maxcount // C_CAP)
    # bf16 matmuls run at 1 col/cycle for any moving size, so the token
    # capacity only needs 8-alignment (16B DMA lines), not 128.
    c = -(-(-(-maxcount // nrounds)) // 8) * 8
    c = max(c, 128)

    w13t, w2t = _retile_weights(w1, w2, w3)
    nc = _get_prog(c)

    xb = x.T.astype(NP_BF16)  # [H, T] once, sliced per expert below
    out = np.zeros((T, H), dtype=np.float32)
    for r in range(nrounds):
        in_maps = []
        seg_toks = []
        seg_wts = []
        for e in range(E):
            seg = toks[e][r * c : (r + 1) * c]
            sw = wts[e][r * c : (r + 1) * c]
            seg_toks.append(seg)
            seg_wts.append(sw)
            xga = np.zeros((H, c), dtype=NP_BF16)
            if len(seg):
                xga[:, : len(seg)] = xb[:, seg]
            in_maps.append(
                {
                    "xgT": np.ascontiguousarray(xga.reshape(HK, 128, c)),
                    "w13": w13t[e],
                    "w2": w2t[e],
                }
            )
        res = run_bass_kernel_spmd(nc, in_maps, core_ids=list(range(E)))
        for e in range(E):
            seg = seg_toks[e]
            if len(seg) == 0:
                continue
            ye = res.results[e]["y"][:, : len(seg)].T.astype(np.float32)
            out[seg] += ye * seg_wts[e][:, None]
    return out
